# revision 19
# baseline (speedup 1.0000x reference)
"""GAT layer kernel for Trainium2, data-parallel over batch across 8 NeuronCores.

Per batch element b (one core each):
    hp  = h @ W_proj + b_proj                      # [N, D]
    s   = hp @ w_src ; t = hp @ w_dst              # [N]
    e   = relu(s[:,None] + t[None,:] + b_att)      # [N, N]
    att = exp(e) * a ; att /= att.sum(-1, keepdim) # [N, N]
    out = att @ hp + hp                            # [N, D]

Design (v2 — transposed-domain, zero on-device transposes):
  * Host marshaling: a is shipped TRANSPOSED and cast to bf16 (aT[j,i] =
    a[i,j]); h is shipped transposed (hT f32). Tiny weights are shipped
    pre-replicated (w_src tiled to 128 cols, b_att to 128 rows).
  * exp(relu(x)) == max(exp(x), 1) and exp(s_i+t_j) == u_i*v_j with
    u = exp(s), v = exp(t + b_att). u is materialized replicated across
    partitions ([128, N] bf16) via a matmul with a column-replicated w_src;
    v lands as a per-partition scalar column ([128, 16]).
  * The score matrix is built directly in TRANSPOSED orientation, per
    128-row j-chunk: PT[j, i] = max(u_i * v_j, 1) * aT[j, i]
    (DVE tensor_scalar 4x-bf16 + tensor_tensor 2x-bf16). No DMA/PE
    transposes of the NxN matrix anywhere.
  * Matmul: PT chunks are the STATIONARY operand, rhs = [hp + b | 1]
    ([128, 129] bf16). Output accumulates NATURALLY as [i, d] in PSUM and
    the row-sums fall out as the free 129th column. 16 accumulators are
    packed 3-per-bank into 6 PSUM banks.
  * Algebra: rhs cols 0:128 carry hp + b_proj; the residual added at the
    end is hp0 = h@W (bias-free). Then
        psum/rowsum + hp0 = P@hp/rs + b_proj + hp0 = P@hp/rs + hp.  (exact)
  * Finalize: one DVE reciprocal + one fused scalar_tensor_tensor per
    i-chunk: out = psum * (1/rowsum) + hp0.
"""

import os
import sys

for _p in ("/opt/trn_rl_repo", "/root/.axon_site/_ro/trn_rl_repo"):
    if _p not in sys.path and os.path.isdir(_p):
        sys.path.append(_p)

import numpy as np
from contextlib import ExitStack

import concourse.bass as bass
import concourse.bacc as bacc
import concourse.tile as tile
from concourse import mybir
from concourse.bass_utils import run_bass_kernel_spmd

F32 = mybir.dt.float32
BF16 = mybir.dt.bfloat16
MULT = mybir.AluOpType.mult
MAX = mybir.AluOpType.max
ADD = mybir.AluOpType.add
EXP = mybir.ActivationFunctionType.Exp
LRELU = mybir.ActivationFunctionType.Lrelu
COPY = mybir.ActivationFunctionType.Copy
IDENT = mybir.ActivationFunctionType.Identity

B, N, D = 8, 2048, 128
P = 128           # partitions
NT = N // P       # 16 chunks
N_CORES = 8


def _build_kernel(ctx: ExitStack, tc: tile.TileContext, io: dict):
    nc = tc.nc
    aT = io["aT"]          # [N, N] bf16 dram: aT[j, i] = a[i, j]
    hT_d = io["hT"]        # [D, N] f32: h transposed (for hp0 f32 matmuls)
    hTb_d = io["hTb"]      # [D, N] bf16: h transposed (for the u matmuls)

    W = io["W_proj"]       # [D, D] f32: W[in, d]
    ws2rb = io["ws2rb"]    # [D, 128] bf16: (W @ w_src) replicated columns
    wd2b = io["wd2b"]      # [D, 1] bf16: W @ w_dst
    cas = io["cas"]        # [128, 1] f32: b_proj @ w_src, replicated
    ba2 = io["ba2"]        # [128, 1] f32: b_proj @ w_dst + b_att, replicated
    out = io["out"]        # [N, D] f32 dram

    cst = ctx.enter_context(tc.tile_pool(name="cst", bufs=1))
    sps = ctx.enter_context(tc.tile_pool(name="sps", bufs=2, space="PSUM"))
    mmp = ctx.enter_context(tc.tile_pool(name="mmp", bufs=1, space="PSUM"))
    zp = ctx.enter_context(tc.tile_pool(name="zp", bufs=1))
    zap = ctx.enter_context(tc.tile_pool(name="zap", bufs=1))
    ppx = ctx.enter_context(tc.tile_pool(name="ppx", bufs=1))
    rp = ctx.enter_context(tc.tile_pool(name="rp", bufs=1))

    # ---- ACT warm-up: trigger the lazy ACT_TABLE_LOAD off the critical path
    warm_in = cst.tile([1, 1], F32)
    nc.vector.memset(warm_in[:], 0.0)
    warm_out = cst.tile([1, 1], F32)
    nc.scalar.activation(warm_out[:], warm_in[:], EXP)

    # ---- critical loads on the Sync HWDGE queue, then the aT flood ----
    hTb_sb = cst.tile([P, N], BF16)
    nc.gpsimd.dma_start(hTb_sb[:], hTb_d[:])
    ws2rb_sb = cst.tile([P, P], BF16)
    nc.gpsimd.dma_start(ws2rb_sb[:], ws2rb[:])
    cas_sb = cst.tile([P, 1], F32)
    nc.gpsimd.dma_start(cas_sb[:], cas[:])
    wd2b_sb = cst.tile([P, 1], BF16)
    nc.gpsimd.dma_start(wd2b_sb[:], wd2b[:])
    W_sb = cst.tile([P, D], F32)
    nc.sync.dma_start(W_sb[:], W[:])
    hT_sb = cst.tile([P, N], F32)
    aT_tiles = []

    def load_aT(jc):
        a_t = cst.tile([P, N], BF16, tag=f"at{jc}", name=f"aT{jc}")
        nc.sync.dma_start(
            a_t[:],
            aT[jc * P:(jc + 1) * P, :].rearrange("(c p) i -> p c i", p=P))
        aT_tiles.append(a_t)

    # interleave the 4 hT quarter-loads with the first aT chunks so neither
    # the first score chunk nor the first hp chunk waits on the other's data
    nc.sync.dma_start(hT_sb[:, 0:512], hT_d[:, 0:512])
    load_aT(0)
    nc.sync.dma_start(hT_sb[:, 512:1024], hT_d[:, 512:1024])
    load_aT(1)
    nc.sync.dma_start(hT_sb[:, 1024:1536], hT_d[:, 1024:1536])
    load_aT(2)
    nc.sync.dma_start(hT_sb[:, 1536:2048], hT_d[:, 1536:2048])
    for jc in range(3, NT):
        load_aT(jc)

    # ---- secondary loads on the Scalar HWDGE queue (parallel issue) ----
    ba2_sb = cst.tile([P, 1], F32)
    nc.scalar.dma_start(ba2_sb[:], ba2[:])

    # ---- u_full [p, i] bf16 = exp(s_i) replicated; s = h @ (W w_src) + cas
    u_full = cst.tile([P, N], BF16)
    for s4 in range(4):
        sl = slice(s4 * 512, (s4 + 1) * 512)
        ps = sps.tile([P, 512], F32, tag="sps")
        nc.tensor.matmul(ps[:], ws2rb_sb[:], hTb_sb[:, sl])
        nc.scalar.activation(u_full[:, sl], ps[:], EXP, bias=cas_sb[:],
                             scale=1.0)

    # ---- t/v: t[jc*128+p] via 1-wide bf16 matmuls on hTb ----
    v_col = cst.tile([P, NT], F32)
    t_ps = sps.tile([P, 512], F32, tag="sps")
    for r in range(NT):
        nc.tensor.matmul(t_ps[:, r:r + 1], hTb_sb[:, r * P:(r + 1) * P],
                         wd2b_sb[:])
        if r == 7:
            nc.scalar.activation(v_col[:, 0:8], t_ps[:, 0:8], EXP,
                                 bias=ba2_sb[:], scale=1.0)
    nc.scalar.activation(v_col[:, 8:NT], t_ps[:, 8:NT], EXP, bias=ba2_sb[:],
                         scale=1.0)

    # ---- hp chunks: hp0 = h @ W (f32), hp_aug = [hp0 | 1] bf16.
    # Algebra: P@hp0/rs + hp0 == P@hp/rs + hp - 2*b_proj; the constant
    # 2*b_proj row is added back on the HOST after the kernel returns. ----
    hp0 = cst.tile([P, NT, D], F32)
    hp_aug = cst.tile([P, NT, 132], BF16)
    nc.vector.memset(hp_aug[:, :, 128:129], 1.0)

    def emit_hp(r):
        if r >= NT:
            return
        ps = sps.tile([P, 512], F32, tag="sps", name="hp_ps")
        nc.tensor.matmul(ps[:, :P], hT_sb[:, r * P:(r + 1) * P], W_sb[:])
        nc.scalar.activation(hp0[:, r, :], ps[:, :P], COPY)
        nc.scalar.activation(hp_aug[:, r, 0:P], ps[:, :P], COPY)

    for r in range(NT):
        emit_hp(r)

    # ---- main psum: 16 accumulators [128, 129], packed 3 per bank ----
    mm_tiles = [mmp.tile([P, 512], F32, tag=f"mm{b}", name=f"mm{b}")
                for b in range(6)]

    def acc(ic):
        bank, slot = divmod(ic, 3)
        off = 130 * slot
        return mm_tiles[bank], off

    out_stage = cst.tile([P, NT, D], F32)

    # ---- main loop over j-chunks; z runs 3 chunks ahead of the product
    # so the pre-p0 DVE idle window does useful z work ----
    z_tiles = {}

    def emit_z(jc):
        if jc >= NT:
            return
        z_t = zp.tile([P, N], BF16, tag=f"z{jc % 3}", name="z_t")
        nc.vector.tensor_scalar(z_t[:], u_full[:], v_col[:, jc:jc + 1],
                                1.0, MULT, MAX)
        z_tiles[jc] = z_t

    for jc in range(3):
        emit_z(jc)
    for jc in range(NT):
        p_t = ppx.tile([P, N], BF16, tag=f"p{jc % 3}", name="p_t")
        nc.vector.tensor_tensor(p_t[:], z_tiles.pop(jc)[:], aT_tiles[jc][:],
                                MULT)
        emit_z(jc + 3)
        for ic in range(NT):
            mt, off = acc(ic)
            # start=True clears PSUM has_written at BANK granularity on
            # TRN2: only the first accumulator of each bank may use it, or
            # it wipes its siblings' jc=0 contribution. The bank-wide clear
            # leaves the sibling slots "unwritten", so their jc=0 matmul
            # (start=False) overwrites rather than accumulates - correct on
            # every execution.
            nc.tensor.matmul(mt[:, off:off + 129],
                             p_t[:, ic * P:(ic + 1) * P],
                             hp_aug[:, jc, 0:129],
                             start=(jc == 0 and ic % 3 == 0),
                             stop=(jc == NT - 1),
                             skip_group_check=True)

    # ---- finalize in 4 groups of 4: out = psum/rowsum + hp0.
    # Two parallel pipelines per group: 2 chunks fused on DVE (stt), 2 on
    # ACT (scaled copy) + Pool (residual add), so no engine chain
    # serializes. ----
    out_r = out.rearrange("(r p) d -> p r d", p=P)
    rinv_all = cst.tile([P, NT], F32)
    for g in range(4):
        ics = list(range(4 * g, 4 * g + 4))
        for ic in ics:
            mt, off = acc(ic)
            nc.vector.reciprocal(rinv_all[:, ic:ic + 1],
                                 mt[:, off + 128:off + 129])
        for ic in ics[2:]:
            mt, off = acc(ic)
            fin = rp.tile([P, D], F32, tag=f"fin{ic % 4}", name="fin")
            nc.scalar.activation(fin[:], mt[:, off:off + 128], COPY,
                                 scale=rinv_all[:, ic:ic + 1])
            nc.gpsimd.tensor_tensor(out_stage[:, ic, :], fin[:],
                                    hp0[:, ic, :], ADD)
        for ic in ics[:2]:
            mt, off = acc(ic)
            nc.vector.scalar_tensor_tensor(out_stage[:, ic, :],
                                           mt[:, off:off + 128],
                                           rinv_all[:, ic:ic + 1],
                                           hp0[:, ic, :], MULT, ADD)
        nc.sync.dma_start(out_r[:, 4 * g:4 * g + 4, :],
                          out_stage[:, 4 * g:4 * g + 4, :])


_CACHE = {}


def _get_compiled():
    if "nc" in _CACHE:
        return _CACHE["nc"], _CACHE["names"]

    nc = bacc.Bacc("TRN2", target_bir_lowering=False, debug=False)
    io = {}
    io["aT"] = nc.dram_tensor("aT", [N, N], BF16, kind="ExternalInput").ap()
    io["hT"] = nc.dram_tensor("hT", [D, N], F32, kind="ExternalInput").ap()
    io["hTb"] = nc.dram_tensor("hTb", [D, N], BF16, kind="ExternalInput").ap()

    io["W_proj"] = nc.dram_tensor("W_proj", [D, D], F32, kind="ExternalInput").ap()
    io["ws2rb"] = nc.dram_tensor("ws2rb", [D, P], BF16, kind="ExternalInput").ap()
    io["wd2b"] = nc.dram_tensor("wd2b", [D, 1], BF16, kind="ExternalInput").ap()
    io["cas"] = nc.dram_tensor("cas", [P, 1], F32, kind="ExternalInput").ap()
    io["ba2"] = nc.dram_tensor("ba2", [P, 1], F32, kind="ExternalInput").ap()
    io["out"] = nc.dram_tensor("out", [N, D], F32, kind="ExternalOutput").ap()

    with tile.TileContext(nc) as tc:
        with ExitStack() as ctx:
            _build_kernel(ctx, tc, io)
    nc.compile()

    _CACHE["nc"] = nc
    _CACHE["names"] = list(io.keys())
    return nc, _CACHE["names"]


def _make_in_maps(a, h, W_proj, b_proj, w_att, b_att):
    import ml_dtypes
    bf16 = ml_dtypes.bfloat16

    a = np.asarray(a, dtype=np.float32)
    h = np.asarray(h, dtype=np.float32)
    W_proj = np.ascontiguousarray(W_proj, dtype=np.float32)
    b_proj = np.asarray(b_proj, dtype=np.float32).reshape(D)
    w_att = np.ascontiguousarray(w_att, dtype=np.float32)
    w_src, w_dst = w_att[:D], w_att[D:]

    ws2 = (W_proj @ w_src).astype(np.float32)
    ws2rb = np.ascontiguousarray(np.tile(ws2[:, None], (1, P))).astype(bf16)
    wd2b = np.ascontiguousarray(
        (W_proj @ w_dst).astype(np.float32)).reshape(D, 1).astype(bf16)
    cas = np.full((P, 1), float(b_proj @ w_src), dtype=np.float32)
    ba2 = np.full((P, 1), float(b_proj @ w_dst) + float(b_att),
                  dtype=np.float32)

    in_maps = []
    for c in range(N_CORES):
        hT_c = np.ascontiguousarray(h[c].T)
        in_maps.append({
            "aT": np.ascontiguousarray(a[c].T).astype(bf16),
            "hT": hT_c, "hTb": hT_c.astype(bf16), "W_proj": W_proj,
            "ws2rb": ws2rb, "wd2b": wd2b, "cas": cas, "ba2": ba2,
        })
    return in_maps


def _get_executable():
    """Build (once) a sharded PJRT callable for the compiled Bass module.

    Mirrors concourse.bass2jax.run_bass_via_pjrt but keeps the jitted
    function so repeated calls don't retrace/recompile.
    """
    if "exe" in _CACHE:
        return _CACHE["exe"]

    import jax
    from jax.sharding import Mesh, PartitionSpec
    from jax.experimental.shard_map import shard_map
    from concourse import bass2jax, mybir as _mybir

    nc, _ = _get_compiled()
    bass2jax.install_neuronx_cc_hook()

    partition_name = (nc.partition_id_tensor.name
                      if nc.partition_id_tensor else None)
    in_names, out_names, out_avals, zero_outs = [], [], [], []
    for alloc in nc.m.functions[0].allocations:
        if not isinstance(alloc, _mybir.MemoryLocationSet):
            continue
        name = alloc.memorylocations[0].name
        if alloc.kind == "ExternalInput":
            if name != partition_name:
                in_names.append(name)
        elif alloc.kind == "ExternalOutput":
            shape = tuple(alloc.tensor_shape)
            dtype = _mybir.dt.np(alloc.dtype)
            out_names.append(name)
            out_avals.append(jax.core.ShapedArray(shape, dtype))
            zero_outs.append(np.zeros(shape, dtype))
    n_params = len(in_names)
    n_outs = len(out_avals)
    all_in_names = in_names + out_names + (
        [partition_name] if partition_name else [])
    donate = tuple(range(n_params, n_params + n_outs))

    def _body(*args):
        operands = list(args)
        if partition_name is not None:
            operands.append(bass2jax.partition_id_tensor())
        outs = bass2jax._bass_exec_p.bind(
            *operands,
            out_avals=tuple(out_avals),
            in_names=tuple(all_in_names),
            out_names=tuple(out_names),
            lowering_input_output_aliases=(),
            sim_require_finite=True,
            sim_require_nnan=True,
            nc=nc,
        )
        return tuple(outs)

    devices = jax.devices()[:N_CORES]
    mesh = Mesh(np.asarray(devices), ("core",))
    in_specs = (PartitionSpec("core"),) * (n_params + n_outs)
    out_specs = (PartitionSpec("core"),) * n_outs
    fn = jax.jit(
        shard_map(_body, mesh=mesh, in_specs=in_specs, out_specs=out_specs,
                  check_rep=False),
        donate_argnums=donate, keep_unused=True,
    )
    exe = {
        "fn": fn, "mesh": mesh, "in_names": in_names,
        "out_names": out_names, "out_avals": out_avals,
        "zero_outs": zero_outs, "n_params": n_params,
    }
    _CACHE["exe"] = exe
    return exe


def _concat_inputs(exe, in_maps):
    return [
        np.concatenate([np.asarray(in_maps[c][name])
                        for c in range(N_CORES)], axis=0)
        for name in exe["in_names"]
    ]


def _concat_zeros(exe):
    return [np.zeros((N_CORES * z.shape[0], *z.shape[1:]), z.dtype)
            for z in exe["zero_outs"]]


def kernel(a, h, W_proj, b_proj, w_att, b_att):
    exe = _get_executable()
    in_maps = _make_in_maps(a, h, W_proj, b_proj, w_att, b_att)
    out_arrs = exe["fn"](*_concat_inputs(exe, in_maps), *_concat_zeros(exe))
    i = exe["out_names"].index("out")
    out = np.asarray(out_arrs[i]).reshape(N_CORES, N, D).copy()
    # the kernel computes P@hp0/rs + hp0 with hp0 = h@W (bias-free); the
    # missing constant row 2*b_proj is added here (exact algebra).
    out += 2.0 * np.asarray(b_proj, dtype=np.float32).reshape(1, 1, D)
    return out


if __name__ == "__main__":
    rng = np.random.default_rng(0)
    a = rng.random((B, N, N), dtype=np.float32)
    h = rng.standard_normal((B, N, D), dtype=np.float32)
    W_proj = (rng.standard_normal((D, D)) / np.sqrt(D)).astype(np.float32)
    b_proj = (rng.standard_normal(D) * 0.01).astype(np.float32)
    w_att = (rng.standard_normal(2 * D) / np.sqrt(2 * D)).astype(np.float32)
    b_att = np.float32(rng.standard_normal() * 0.01)

    got = kernel(a=a, h=h, W_proj=W_proj, b_proj=b_proj, w_att=w_att,
                 b_att=b_att)

    hp = h @ W_proj + b_proj
    s = hp @ w_att[:D]
    t = hp @ w_att[D:]
    e = np.maximum(s[:, :, None] + t[:, None, :] + b_att, 0.0)
    att = np.exp(e) * a
    att = att / att.sum(-1, keepdims=True)
    ref = att @ hp + hp

    err = np.abs(got - ref).max() / np.abs(ref).max()
    print("rel err:", err)


# revision 20
# speedup vs baseline: 1.0384x; 1.0384x over previous
"""GAT layer kernel for Trainium2, data-parallel over batch across 8 NeuronCores.

Per batch element b (one core each):
    hp  = h @ W_proj + b_proj                      # [N, D]
    s   = hp @ w_src ; t = hp @ w_dst              # [N]
    e   = relu(s[:,None] + t[None,:] + b_att)      # [N, N]
    att = exp(e) * a ; att /= att.sum(-1, keepdim) # [N, N]
    out = att @ hp + hp                            # [N, D]

Design (v2 — transposed-domain, zero on-device transposes):
  * Host marshaling: a is shipped TRANSPOSED and cast to bf16 (aT[j,i] =
    a[i,j]); h is shipped transposed (hT f32). Tiny weights are shipped
    pre-replicated (w_src tiled to 128 cols, b_att to 128 rows).
  * exp(relu(x)) == max(exp(x), 1) and exp(s_i+t_j) == u_i*v_j with
    u = exp(s), v = exp(t + b_att). u is materialized replicated across
    partitions ([128, N] bf16) via a matmul with a column-replicated w_src;
    v lands as a per-partition scalar column ([128, 16]).
  * The score matrix is built directly in TRANSPOSED orientation, per
    128-row j-chunk: PT[j, i] = max(u_i * v_j, 1) * aT[j, i]
    (DVE tensor_scalar 4x-bf16 + tensor_tensor 2x-bf16). No DMA/PE
    transposes of the NxN matrix anywhere.
  * Matmul: PT chunks are the STATIONARY operand, rhs = [hp + b | 1]
    ([128, 129] bf16). Output accumulates NATURALLY as [i, d] in PSUM and
    the row-sums fall out as the free 129th column. 16 accumulators are
    packed 3-per-bank into 6 PSUM banks.
  * Algebra: rhs cols 0:128 carry hp + b_proj; the residual added at the
    end is hp0 = h@W (bias-free). Then
        psum/rowsum + hp0 = P@hp/rs + b_proj + hp0 = P@hp/rs + hp.  (exact)
  * Finalize: one DVE reciprocal + one fused scalar_tensor_tensor per
    i-chunk: out = psum * (1/rowsum) + hp0.
"""

import os
import sys

for _p in ("/opt/trn_rl_repo", "/root/.axon_site/_ro/trn_rl_repo"):
    if _p not in sys.path and os.path.isdir(_p):
        sys.path.append(_p)

import numpy as np
from contextlib import ExitStack

import concourse.bass as bass
import concourse.bacc as bacc
import concourse.tile as tile
from concourse import mybir
from concourse.bass_utils import run_bass_kernel_spmd

F32 = mybir.dt.float32
BF16 = mybir.dt.bfloat16
MULT = mybir.AluOpType.mult
MAX = mybir.AluOpType.max
ADD = mybir.AluOpType.add
EXP = mybir.ActivationFunctionType.Exp
LRELU = mybir.ActivationFunctionType.Lrelu
COPY = mybir.ActivationFunctionType.Copy
IDENT = mybir.ActivationFunctionType.Identity

B, N, D = 8, 2048, 128
P = 128           # partitions
NT = N // P       # 16 chunks
N_CORES = 8


def _build_kernel(ctx: ExitStack, tc: tile.TileContext, io: dict):
    nc = tc.nc
    aT = io["aT"]          # [N, N] bf16 dram: aT[j, i] = a[i, j]
    hT_d = io["hT"]        # [D, N] f32: h transposed (for hp0 f32 matmuls)
    hTb_d = io["hTb"]      # [D, N] bf16: h transposed (for the u matmuls)

    W = io["W_proj"]       # [D, D] f32: W[in, d]
    ws2rb = io["ws2rb"]    # [D, 128] bf16: (W @ w_src) replicated columns
    wd2b = io["wd2b"]      # [D, 1] bf16: W @ w_dst
    cas = io["cas"]        # [128, 1] f32: b_proj @ w_src, replicated
    ba2 = io["ba2"]        # [128, 1] f32: b_proj @ w_dst + b_att, replicated
    out = io["out"]        # [N, D] f32 dram

    cst = ctx.enter_context(tc.tile_pool(name="cst", bufs=1))
    sps = ctx.enter_context(tc.tile_pool(name="sps", bufs=2, space="PSUM"))
    mmp = ctx.enter_context(tc.tile_pool(name="mmp", bufs=1, space="PSUM"))
    zp = ctx.enter_context(tc.tile_pool(name="zp", bufs=1))
    zap = ctx.enter_context(tc.tile_pool(name="zap", bufs=1))
    ppx = ctx.enter_context(tc.tile_pool(name="ppx", bufs=1))
    rp = ctx.enter_context(tc.tile_pool(name="rp", bufs=1))

    # ---- ACT warm-up: trigger the lazy ACT_TABLE_LOAD off the critical path
    warm_in = cst.tile([1, 1], F32)
    nc.vector.memset(warm_in[:], 0.0)
    warm_out = cst.tile([1, 1], F32)
    nc.scalar.activation(warm_out[:], warm_in[:], EXP)

    # ---- critical loads on the Sync HWDGE queue, then the aT flood ----
    hTb_sb = cst.tile([P, N], BF16)
    nc.sync.dma_start(hTb_sb[:], hTb_d[:])
    ws2rb_sb = cst.tile([P, P], BF16)
    nc.sync.dma_start(ws2rb_sb[:], ws2rb[:])
    cas_sb = cst.tile([P, 1], F32)
    nc.sync.dma_start(cas_sb[:], cas[:])
    wd2b_sb = cst.tile([P, 1], BF16)
    nc.sync.dma_start(wd2b_sb[:], wd2b[:])
    W_sb = cst.tile([P, D], F32)
    nc.sync.dma_start(W_sb[:], W[:])
    hT_sb = cst.tile([P, N], F32)
    aT_tiles = []

    def load_aT(jc):
        a_t = cst.tile([P, N], BF16, tag=f"at{jc}", name=f"aT{jc}")
        nc.sync.dma_start(
            a_t[:],
            aT[jc * P:(jc + 1) * P, :].rearrange("(c p) i -> p c i", p=P))
        aT_tiles.append(a_t)

    # interleave the 4 hT quarter-loads with the first aT chunks so neither
    # the first score chunk nor the first hp chunk waits on the other's data
    nc.sync.dma_start(hT_sb[:, 0:512], hT_d[:, 0:512])
    load_aT(0)
    nc.sync.dma_start(hT_sb[:, 512:1024], hT_d[:, 512:1024])
    load_aT(1)
    nc.sync.dma_start(hT_sb[:, 1024:1536], hT_d[:, 1024:1536])
    load_aT(2)
    nc.sync.dma_start(hT_sb[:, 1536:2048], hT_d[:, 1536:2048])
    for jc in range(3, NT):
        load_aT(jc)

    # ---- secondary loads on the Scalar HWDGE queue (parallel issue) ----
    ba2_sb = cst.tile([P, 1], F32)
    nc.scalar.dma_start(ba2_sb[:], ba2[:])

    # ---- u_full [p, i] bf16 = exp(s_i) replicated; s = h @ (W w_src) + cas
    u_full = cst.tile([P, N], BF16)
    for s4 in range(4):
        sl = slice(s4 * 512, (s4 + 1) * 512)
        ps = sps.tile([P, 512], F32, tag="sps")
        nc.tensor.matmul(ps[:], ws2rb_sb[:], hTb_sb[:, sl])
        nc.scalar.activation(u_full[:, sl], ps[:], EXP, bias=cas_sb[:],
                             scale=1.0)

    # ---- t/v: t[jc*128+p] via 1-wide bf16 matmuls on hTb ----
    v_col = cst.tile([P, NT], F32)
    t_ps = sps.tile([P, 512], F32, tag="sps")
    for r in range(NT):
        nc.tensor.matmul(t_ps[:, r:r + 1], hTb_sb[:, r * P:(r + 1) * P],
                         wd2b_sb[:])
        if r == 7:
            nc.scalar.activation(v_col[:, 0:8], t_ps[:, 0:8], EXP,
                                 bias=ba2_sb[:], scale=1.0)
    nc.scalar.activation(v_col[:, 8:NT], t_ps[:, 8:NT], EXP, bias=ba2_sb[:],
                         scale=1.0)

    # ---- hp chunks: hp0 = h @ W (f32), hp_aug = [hp0 | 1] bf16.
    # Algebra: P@hp0/rs + hp0 == P@hp/rs + hp - 2*b_proj; the constant
    # 2*b_proj row is added back on the HOST after the kernel returns. ----
    hp0 = cst.tile([P, NT, D], F32)
    hp_aug = cst.tile([P, NT, 132], BF16)
    nc.vector.memset(hp_aug[:, :, 128:129], 1.0)

    def emit_hp(r):
        if r >= NT:
            return
        ps = sps.tile([P, 512], F32, tag="sps", name="hp_ps")
        nc.tensor.matmul(ps[:, :P], hT_sb[:, r * P:(r + 1) * P], W_sb[:])
        nc.scalar.activation(hp0[:, r, :], ps[:, :P], COPY)
        nc.scalar.activation(hp_aug[:, r, 0:P], ps[:, :P], COPY)

    for r in range(NT):
        emit_hp(r)

    # ---- main psum: 16 accumulators [128, 129], packed 3 per bank ----
    mm_tiles = [mmp.tile([P, 512], F32, tag=f"mm{b}", name=f"mm{b}")
                for b in range(6)]

    def acc(ic):
        bank, slot = divmod(ic, 3)
        off = 130 * slot
        return mm_tiles[bank], off

    out_stage = cst.tile([P, NT, D], F32)

    # ---- main loop over j-chunks; z runs 3 chunks ahead of the product
    # so the pre-p0 DVE idle window does useful z work ----
    z_tiles = {}

    def emit_z(jc):
        if jc >= NT:
            return
        z_t = zp.tile([P, N], BF16, tag=f"z{jc % 3}", name="z_t")
        nc.vector.tensor_scalar(z_t[:], u_full[:], v_col[:, jc:jc + 1],
                                1.0, MULT, MAX)
        z_tiles[jc] = z_t

    for jc in range(3):
        emit_z(jc)
    for jc in range(NT):
        p_t = ppx.tile([P, N], BF16, tag=f"p{jc % 3}", name="p_t")
        nc.vector.tensor_tensor(p_t[:], z_tiles.pop(jc)[:], aT_tiles[jc][:],
                                MULT)
        emit_z(jc + 3)
        for ic in range(NT):
            mt, off = acc(ic)
            # start=True clears PSUM has_written at BANK granularity on
            # TRN2: only the first accumulator of each bank may use it, or
            # it wipes its siblings' jc=0 contribution. The bank-wide clear
            # leaves the sibling slots "unwritten", so their jc=0 matmul
            # (start=False) overwrites rather than accumulates - correct on
            # every execution.
            nc.tensor.matmul(mt[:, off:off + 129],
                             p_t[:, ic * P:(ic + 1) * P],
                             hp_aug[:, jc, 0:129],
                             start=(jc == 0 and ic % 3 == 0),
                             stop=(jc == NT - 1),
                             skip_group_check=True)

    # ---- finalize in 4 groups of 4: out = psum/rowsum + hp0.
    # Two parallel pipelines per group: 2 chunks fused on DVE (stt), 2 on
    # ACT (scaled copy) + Pool (residual add), so no engine chain
    # serializes. ----
    out_r = out.rearrange("(r p) d -> p r d", p=P)
    rinv_all = cst.tile([P, NT], F32)
    for g in range(4):
        ics = list(range(4 * g, 4 * g + 4))
        for ic in ics:
            mt, off = acc(ic)
            nc.vector.reciprocal(rinv_all[:, ic:ic + 1],
                                 mt[:, off + 128:off + 129])
        for ic in ics[2:]:
            mt, off = acc(ic)
            fin = rp.tile([P, D], F32, tag=f"fin{ic % 4}", name="fin")
            nc.scalar.activation(fin[:], mt[:, off:off + 128], COPY,
                                 scale=rinv_all[:, ic:ic + 1])
            nc.gpsimd.tensor_tensor(out_stage[:, ic, :], fin[:],
                                    hp0[:, ic, :], ADD)
        for ic in ics[:2]:
            mt, off = acc(ic)
            nc.vector.scalar_tensor_tensor(out_stage[:, ic, :],
                                           mt[:, off:off + 128],
                                           rinv_all[:, ic:ic + 1],
                                           hp0[:, ic, :], MULT, ADD)
        nc.sync.dma_start(out_r[:, 4 * g:4 * g + 4, :],
                          out_stage[:, 4 * g:4 * g + 4, :])


_CACHE = {}


def _get_compiled():
    if "nc" in _CACHE:
        return _CACHE["nc"], _CACHE["names"]

    nc = bacc.Bacc("TRN2", target_bir_lowering=False, debug=False)
    io = {}
    io["aT"] = nc.dram_tensor("aT", [N, N], BF16, kind="ExternalInput").ap()
    io["hT"] = nc.dram_tensor("hT", [D, N], F32, kind="ExternalInput").ap()
    io["hTb"] = nc.dram_tensor("hTb", [D, N], BF16, kind="ExternalInput").ap()

    io["W_proj"] = nc.dram_tensor("W_proj", [D, D], F32, kind="ExternalInput").ap()
    io["ws2rb"] = nc.dram_tensor("ws2rb", [D, P], BF16, kind="ExternalInput").ap()
    io["wd2b"] = nc.dram_tensor("wd2b", [D, 1], BF16, kind="ExternalInput").ap()
    io["cas"] = nc.dram_tensor("cas", [P, 1], F32, kind="ExternalInput").ap()
    io["ba2"] = nc.dram_tensor("ba2", [P, 1], F32, kind="ExternalInput").ap()
    io["out"] = nc.dram_tensor("out", [N, D], F32, kind="ExternalOutput").ap()

    with tile.TileContext(nc) as tc:
        with ExitStack() as ctx:
            _build_kernel(ctx, tc, io)
    nc.compile()

    _CACHE["nc"] = nc
    _CACHE["names"] = list(io.keys())
    return nc, _CACHE["names"]


def _make_in_maps(a, h, W_proj, b_proj, w_att, b_att):
    import ml_dtypes
    bf16 = ml_dtypes.bfloat16

    a = np.asarray(a, dtype=np.float32)
    h = np.asarray(h, dtype=np.float32)
    W_proj = np.ascontiguousarray(W_proj, dtype=np.float32)
    b_proj = np.asarray(b_proj, dtype=np.float32).reshape(D)
    w_att = np.ascontiguousarray(w_att, dtype=np.float32)
    w_src, w_dst = w_att[:D], w_att[D:]

    ws2 = (W_proj @ w_src).astype(np.float32)
    ws2rb = np.ascontiguousarray(np.tile(ws2[:, None], (1, P))).astype(bf16)
    wd2b = np.ascontiguousarray(
        (W_proj @ w_dst).astype(np.float32)).reshape(D, 1).astype(bf16)
    cas = np.full((P, 1), float(b_proj @ w_src), dtype=np.float32)
    ba2 = np.full((P, 1), float(b_proj @ w_dst) + float(b_att),
                  dtype=np.float32)

    in_maps = []
    for c in range(N_CORES):
        hT_c = np.ascontiguousarray(h[c].T)
        in_maps.append({
            "aT": np.ascontiguousarray(a[c].T).astype(bf16),
            "hT": hT_c, "hTb": hT_c.astype(bf16), "W_proj": W_proj,
            "ws2rb": ws2rb, "wd2b": wd2b, "cas": cas, "ba2": ba2,
        })
    return in_maps


def _get_executable():
    """Build (once) a sharded PJRT callable for the compiled Bass module.

    Mirrors concourse.bass2jax.run_bass_via_pjrt but keeps the jitted
    function so repeated calls don't retrace/recompile.
    """
    if "exe" in _CACHE:
        return _CACHE["exe"]

    import jax
    from jax.sharding import Mesh, PartitionSpec
    from jax.experimental.shard_map import shard_map
    from concourse import bass2jax, mybir as _mybir

    nc, _ = _get_compiled()
    bass2jax.install_neuronx_cc_hook()

    partition_name = (nc.partition_id_tensor.name
                      if nc.partition_id_tensor else None)
    in_names, out_names, out_avals, zero_outs = [], [], [], []
    for alloc in nc.m.functions[0].allocations:
        if not isinstance(alloc, _mybir.MemoryLocationSet):
            continue
        name = alloc.memorylocations[0].name
        if alloc.kind == "ExternalInput":
            if name != partition_name:
                in_names.append(name)
        elif alloc.kind == "ExternalOutput":
            shape = tuple(alloc.tensor_shape)
            dtype = _mybir.dt.np(alloc.dtype)
            out_names.append(name)
            out_avals.append(jax.core.ShapedArray(shape, dtype))
            zero_outs.append(np.zeros(shape, dtype))
    n_params = len(in_names)
    n_outs = len(out_avals)
    all_in_names = in_names + out_names + (
        [partition_name] if partition_name else [])
    donate = tuple(range(n_params, n_params + n_outs))

    def _body(*args):
        operands = list(args)
        if partition_name is not None:
            operands.append(bass2jax.partition_id_tensor())
        outs = bass2jax._bass_exec_p.bind(
            *operands,
            out_avals=tuple(out_avals),
            in_names=tuple(all_in_names),
            out_names=tuple(out_names),
            lowering_input_output_aliases=(),
            sim_require_finite=True,
            sim_require_nnan=True,
            nc=nc,
        )
        return tuple(outs)

    devices = jax.devices()[:N_CORES]
    mesh = Mesh(np.asarray(devices), ("core",))
    in_specs = (PartitionSpec("core"),) * (n_params + n_outs)
    out_specs = (PartitionSpec("core"),) * n_outs
    fn = jax.jit(
        shard_map(_body, mesh=mesh, in_specs=in_specs, out_specs=out_specs,
                  check_rep=False),
        donate_argnums=donate, keep_unused=True,
    )
    exe = {
        "fn": fn, "mesh": mesh, "in_names": in_names,
        "out_names": out_names, "out_avals": out_avals,
        "zero_outs": zero_outs, "n_params": n_params,
    }
    _CACHE["exe"] = exe
    return exe


def _concat_inputs(exe, in_maps):
    return [
        np.concatenate([np.asarray(in_maps[c][name])
                        for c in range(N_CORES)], axis=0)
        for name in exe["in_names"]
    ]


def _concat_zeros(exe):
    return [np.zeros((N_CORES * z.shape[0], *z.shape[1:]), z.dtype)
            for z in exe["zero_outs"]]


def kernel(a, h, W_proj, b_proj, w_att, b_att):
    exe = _get_executable()
    in_maps = _make_in_maps(a, h, W_proj, b_proj, w_att, b_att)
    out_arrs = exe["fn"](*_concat_inputs(exe, in_maps), *_concat_zeros(exe))
    i = exe["out_names"].index("out")
    out = np.asarray(out_arrs[i]).reshape(N_CORES, N, D).copy()
    # the kernel computes P@hp0/rs + hp0 with hp0 = h@W (bias-free); the
    # missing constant row 2*b_proj is added here (exact algebra).
    out += 2.0 * np.asarray(b_proj, dtype=np.float32).reshape(1, 1, D)
    return out


if __name__ == "__main__":
    rng = np.random.default_rng(0)
    a = rng.random((B, N, N), dtype=np.float32)
    h = rng.standard_normal((B, N, D), dtype=np.float32)
    W_proj = (rng.standard_normal((D, D)) / np.sqrt(D)).astype(np.float32)
    b_proj = (rng.standard_normal(D) * 0.01).astype(np.float32)
    w_att = (rng.standard_normal(2 * D) / np.sqrt(2 * D)).astype(np.float32)
    b_att = np.float32(rng.standard_normal() * 0.01)

    got = kernel(a=a, h=h, W_proj=W_proj, b_proj=b_proj, w_att=w_att,
                 b_att=b_att)

    hp = h @ W_proj + b_proj
    s = hp @ w_att[:D]
    t = hp @ w_att[D:]
    e = np.maximum(s[:, :, None] + t[:, None, :] + b_att, 0.0)
    att = np.exp(e) * a
    att = att / att.sum(-1, keepdims=True)
    ref = att @ hp + hp

    err = np.abs(got - ref).max() / np.abs(ref).max()
    print("rel err:", err)


# revision 21
# speedup vs baseline: 1.0713x; 1.0317x over previous
"""GAT layer kernel for Trainium2, data-parallel over batch across 8 NeuronCores.

Per batch element b (one core each):
    hp  = h @ W_proj + b_proj                      # [N, D]
    s   = hp @ w_src ; t = hp @ w_dst              # [N]
    e   = relu(s[:,None] + t[None,:] + b_att)      # [N, N]
    att = exp(e) * a ; att /= att.sum(-1, keepdim) # [N, N]
    out = att @ hp + hp                            # [N, D]

Design (v2 — transposed-domain, zero on-device transposes):
  * Host marshaling: a is shipped TRANSPOSED and cast to bf16 (aT[j,i] =
    a[i,j]); h is shipped transposed (hT f32). Tiny weights are shipped
    pre-replicated (w_src tiled to 128 cols, b_att to 128 rows).
  * exp(relu(x)) == max(exp(x), 1) and exp(s_i+t_j) == u_i*v_j with
    u = exp(s), v = exp(t + b_att). u is materialized replicated across
    partitions ([128, N] bf16) via a matmul with a column-replicated w_src;
    v lands as a per-partition scalar column ([128, 16]).
  * The score matrix is built directly in TRANSPOSED orientation, per
    128-row j-chunk: PT[j, i] = max(u_i * v_j, 1) * aT[j, i]
    (DVE tensor_scalar 4x-bf16 + tensor_tensor 2x-bf16). No DMA/PE
    transposes of the NxN matrix anywhere.
  * Matmul: PT chunks are the STATIONARY operand, rhs = [hp + b | 1]
    ([128, 129] bf16). Output accumulates NATURALLY as [i, d] in PSUM and
    the row-sums fall out as the free 129th column. 16 accumulators are
    packed 3-per-bank into 6 PSUM banks.
  * Algebra: rhs cols 0:128 carry hp + b_proj; the residual added at the
    end is hp0 = h@W (bias-free). Then
        psum/rowsum + hp0 = P@hp/rs + b_proj + hp0 = P@hp/rs + hp.  (exact)
  * Finalize: one DVE reciprocal + one fused scalar_tensor_tensor per
    i-chunk: out = psum * (1/rowsum) + hp0.
"""

import os
import sys

for _p in ("/opt/trn_rl_repo", "/root/.axon_site/_ro/trn_rl_repo"):
    if _p not in sys.path and os.path.isdir(_p):
        sys.path.append(_p)

import numpy as np
from contextlib import ExitStack

import concourse.bass as bass
import concourse.bacc as bacc
import concourse.tile as tile
from concourse import mybir
from concourse.bass_utils import run_bass_kernel_spmd

F32 = mybir.dt.float32
BF16 = mybir.dt.bfloat16
MULT = mybir.AluOpType.mult
MAX = mybir.AluOpType.max
ADD = mybir.AluOpType.add
EXP = mybir.ActivationFunctionType.Exp
LRELU = mybir.ActivationFunctionType.Lrelu
COPY = mybir.ActivationFunctionType.Copy
IDENT = mybir.ActivationFunctionType.Identity

B, N, D = 8, 2048, 128
P = 128           # partitions
NT = N // P       # 16 chunks
N_CORES = 8


def _build_kernel(ctx: ExitStack, tc: tile.TileContext, io: dict):
    nc = tc.nc
    aT = io["aT"]          # [N, N] bf16 dram: aT[j, i] = a[i, j]
    hT_d = io["hT"]        # [D, N] f32: h transposed (for hp0 f32 matmuls)
    hx_d = io["hx"]        # [D, N+129] bf16: [hT | ws2r | wd2] packed

    W = io["W_proj"]       # [D, D] f32: W[in, d]


    cas = io["cas"]        # [128, 1] f32: b_proj @ w_src, replicated
    ba2 = io["ba2"]        # [128, 1] f32: b_proj @ w_dst + b_att, replicated
    out = io["out"]        # [N, D] f32 dram

    cst = ctx.enter_context(tc.tile_pool(name="cst", bufs=1))
    sps = ctx.enter_context(tc.tile_pool(name="sps", bufs=2, space="PSUM"))
    mmp = ctx.enter_context(tc.tile_pool(name="mmp", bufs=1, space="PSUM"))
    zp = ctx.enter_context(tc.tile_pool(name="zp", bufs=1))
    zap = ctx.enter_context(tc.tile_pool(name="zap", bufs=1))
    ppx = ctx.enter_context(tc.tile_pool(name="ppx", bufs=1))
    rp = ctx.enter_context(tc.tile_pool(name="rp", bufs=1))

    # ---- ACT warm-up: trigger the lazy ACT_TABLE_LOAD off the critical path
    warm_in = cst.tile([1, 1], F32)
    nc.vector.memset(warm_in[:], 0.0)
    warm_out = cst.tile([1, 1], F32)
    nc.scalar.activation(warm_out[:], warm_in[:], EXP)

    # ---- critical loads on the Sync HWDGE queue, then the aT flood ----
    hx_sb = cst.tile([P, N + 129], BF16)
    nc.sync.dma_start(hx_sb[:], hx_d[:])
    hTb_sb = hx_sb[:, 0:N]
    ws2rb_sb = hx_sb[:, N:N + 128]
    wd2b_sb = hx_sb[:, N + 128:N + 129]
    cas_sb = cst.tile([P, 1], F32)
    nc.sync.dma_start(cas_sb[:], cas[:])
    W_sb = cst.tile([P, D], F32)
    nc.sync.dma_start(W_sb[:], W[:])
    hT_sb = cst.tile([P, N], F32)
    aT_tiles = []

    def load_aT(jc):
        a_t = cst.tile([P, N], BF16, tag=f"at{jc}", name=f"aT{jc}")
        nc.sync.dma_start(
            a_t[:],
            aT[jc * P:(jc + 1) * P, :].rearrange("(c p) i -> p c i", p=P))
        aT_tiles.append(a_t)

    # interleave the 4 hT quarter-loads with the first aT chunks so neither
    # the first score chunk nor the first hp chunk waits on the other's data
    nc.sync.dma_start(hT_sb[:, 0:512], hT_d[:, 0:512])
    load_aT(0)
    nc.sync.dma_start(hT_sb[:, 512:1024], hT_d[:, 512:1024])
    load_aT(1)
    nc.sync.dma_start(hT_sb[:, 1024:1536], hT_d[:, 1024:1536])
    load_aT(2)
    nc.sync.dma_start(hT_sb[:, 1536:2048], hT_d[:, 1536:2048])
    for jc in range(3, NT):
        load_aT(jc)

    # ---- secondary loads on the Scalar HWDGE queue (parallel issue) ----
    ba2_sb = cst.tile([P, 1], F32)
    nc.scalar.dma_start(ba2_sb[:], ba2[:])

    # ---- u_full [p, i] bf16 = exp(s_i) replicated; s = h @ (W w_src) + cas
    u_full = cst.tile([P, N], BF16)
    for s4 in range(4):
        sl = slice(s4 * 512, (s4 + 1) * 512)
        ps = sps.tile([P, 512], F32, tag="sps")
        nc.tensor.matmul(ps[:], ws2rb_sb, hTb_sb[:, sl])
        nc.scalar.activation(u_full[:, sl], ps[:], EXP, bias=cas_sb[:],
                             scale=1.0)

    # ---- t/v: t[jc*128+p] via 1-wide bf16 matmuls on hTb ----
    v_col = cst.tile([P, NT], F32)
    t_ps = sps.tile([P, 512], F32, tag="sps")
    for r in range(NT):
        nc.tensor.matmul(t_ps[:, r:r + 1], hTb_sb[:, r * P:(r + 1) * P],
                         wd2b_sb)
        if r == 7:
            nc.scalar.activation(v_col[:, 0:8], t_ps[:, 0:8], EXP,
                                 bias=ba2_sb[:], scale=1.0)
    nc.scalar.activation(v_col[:, 8:NT], t_ps[:, 8:NT], EXP, bias=ba2_sb[:],
                         scale=1.0)

    # ---- hp chunks: hp0 = h @ W (f32), hp_aug = [hp0 | 1] bf16.
    # Algebra: P@hp0/rs + hp0 == P@hp/rs + hp - 2*b_proj; the constant
    # 2*b_proj row is added back on the HOST after the kernel returns. ----
    hp_aug = cst.tile([P, NT, 132], BF16)
    nc.vector.memset(hp_aug[:, :, 128:129], 1.0)

    def emit_hp(r):
        if r >= NT:
            return
        ps = sps.tile([P, 512], F32, tag="sps", name="hp_ps")
        nc.tensor.matmul(ps[:, :P], hT_sb[:, r * P:(r + 1) * P], W_sb[:])
        nc.scalar.activation(hp_aug[:, r, 0:P], ps[:, :P], COPY)

    for r in range(NT):
        emit_hp(r)

    # ---- main psum: 16 accumulators [128, 129], packed 3 per bank ----
    mm_tiles = [mmp.tile([P, 512], F32, tag=f"mm{b}", name=f"mm{b}")
                for b in range(6)]

    def acc(ic):
        bank, slot = divmod(ic, 3)
        off = 130 * slot
        return mm_tiles[bank], off

    out_stage = cst.tile([P, NT, D], F32)

    # ---- main loop over j-chunks; z runs 3 chunks ahead of the product
    # so the pre-p0 DVE idle window does useful z work ----
    z_tiles = {}

    def emit_z(jc):
        if jc >= NT:
            return
        z_t = zp.tile([P, N], BF16, tag=f"z{jc % 3}", name="z_t")
        nc.vector.tensor_scalar(z_t[:], u_full[:], v_col[:, jc:jc + 1],
                                1.0, MULT, MAX)
        z_tiles[jc] = z_t

    for jc in range(3):
        emit_z(jc)
    for jc in range(NT):
        p_t = ppx.tile([P, N], BF16, tag=f"p{jc % 3}", name="p_t")
        nc.vector.tensor_tensor(p_t[:], z_tiles.pop(jc)[:], aT_tiles[jc][:],
                                MULT)
        emit_z(jc + 3)
        for ic in range(NT):
            mt, off = acc(ic)
            # start=True clears PSUM has_written at BANK granularity on
            # TRN2: only the first accumulator of each bank may use it, or
            # it wipes its siblings' jc=0 contribution. The bank-wide clear
            # leaves the sibling slots "unwritten", so their jc=0 matmul
            # (start=False) overwrites rather than accumulates - correct on
            # every execution.
            nc.tensor.matmul(mt[:, off:off + 129],
                             p_t[:, ic * P:(ic + 1) * P],
                             hp_aug[:, jc, 0:129],
                             start=(jc == 0 and ic % 3 == 0),
                             stop=(jc == NT - 1),
                             skip_group_check=True)

    # ---- finalize in 4 groups of 4: out = psum/rowsum + hp0.
    # Two parallel pipelines per group: 2 chunks fused on DVE (stt), 2 on
    # ACT (scaled copy) + Pool (residual add), so no engine chain
    # serializes. ----
    out_r = out.rearrange("(r p) d -> p r d", p=P)
    rinv_all = cst.tile([P, NT], F32)
    for g in range(4):
        ics = list(range(4 * g, 4 * g + 4))
        for ic in ics:
            mt, off = acc(ic)
            nc.vector.reciprocal(rinv_all[:, ic:ic + 1],
                                 mt[:, off + 128:off + 129])
        for ic in ics[2:]:
            mt, off = acc(ic)
            fin = rp.tile([P, D], F32, tag=f"fin{ic % 4}", name="fin")
            nc.scalar.activation(fin[:], mt[:, off:off + 128], COPY,
                                 scale=rinv_all[:, ic:ic + 1])
            nc.gpsimd.tensor_tensor(out_stage[:, ic, :], fin[:],
                                    hp_aug[:, ic, 0:P], ADD)
        for ic in ics[:2]:
            mt, off = acc(ic)
            nc.vector.scalar_tensor_tensor(out_stage[:, ic, :],
                                           mt[:, off:off + 128],
                                           rinv_all[:, ic:ic + 1],
                                           hp_aug[:, ic, 0:P], MULT, ADD)
        nc.sync.dma_start(out_r[:, 4 * g:4 * g + 4, :],
                          out_stage[:, 4 * g:4 * g + 4, :])


_CACHE = {}


def _get_compiled():
    if "nc" in _CACHE:
        return _CACHE["nc"], _CACHE["names"]

    nc = bacc.Bacc("TRN2", target_bir_lowering=False, debug=False)
    io = {}
    io["aT"] = nc.dram_tensor("aT", [N, N], BF16, kind="ExternalInput").ap()
    io["hT"] = nc.dram_tensor("hT", [D, N], F32, kind="ExternalInput").ap()
    io["hx"] = nc.dram_tensor("hx", [D, N + 129], BF16, kind="ExternalInput").ap()

    io["W_proj"] = nc.dram_tensor("W_proj", [D, D], F32, kind="ExternalInput").ap()
    io["cas"] = nc.dram_tensor("cas", [P, 1], F32, kind="ExternalInput").ap()
    io["ba2"] = nc.dram_tensor("ba2", [P, 1], F32, kind="ExternalInput").ap()
    io["out"] = nc.dram_tensor("out", [N, D], F32, kind="ExternalOutput").ap()

    with tile.TileContext(nc) as tc:
        with ExitStack() as ctx:
            _build_kernel(ctx, tc, io)
    nc.compile()

    _CACHE["nc"] = nc
    _CACHE["names"] = list(io.keys())
    return nc, _CACHE["names"]


def _make_in_maps(a, h, W_proj, b_proj, w_att, b_att):
    import ml_dtypes
    bf16 = ml_dtypes.bfloat16

    a = np.asarray(a, dtype=np.float32)
    h = np.asarray(h, dtype=np.float32)
    W_proj = np.ascontiguousarray(W_proj, dtype=np.float32)
    b_proj = np.asarray(b_proj, dtype=np.float32).reshape(D)
    w_att = np.ascontiguousarray(w_att, dtype=np.float32)
    w_src, w_dst = w_att[:D], w_att[D:]

    ws2 = (W_proj @ w_src).astype(np.float32)
    ws2rb = np.tile(ws2[:, None], (1, P)).astype(bf16)
    wd2b = (W_proj @ w_dst).astype(np.float32).reshape(D, 1).astype(bf16)
    cas = np.full((P, 1), float(b_proj @ w_src), dtype=np.float32)
    ba2 = np.full((P, 1), float(b_proj @ w_dst) + float(b_att),
                  dtype=np.float32)

    in_maps = []
    for c in range(N_CORES):
        hT_c = np.ascontiguousarray(h[c].T)
        in_maps.append({
            "aT": np.ascontiguousarray(a[c].T).astype(bf16),
            "hT": hT_c,
            "hx": np.ascontiguousarray(np.concatenate(
                [hT_c.astype(bf16), ws2rb, wd2b], axis=1)),
            "W_proj": W_proj, "cas": cas, "ba2": ba2,
        })
    return in_maps


def _get_executable():
    """Build (once) a sharded PJRT callable for the compiled Bass module.

    Mirrors concourse.bass2jax.run_bass_via_pjrt but keeps the jitted
    function so repeated calls don't retrace/recompile.
    """
    if "exe" in _CACHE:
        return _CACHE["exe"]

    import jax
    from jax.sharding import Mesh, PartitionSpec
    from jax.experimental.shard_map import shard_map
    from concourse import bass2jax, mybir as _mybir

    nc, _ = _get_compiled()
    bass2jax.install_neuronx_cc_hook()

    partition_name = (nc.partition_id_tensor.name
                      if nc.partition_id_tensor else None)
    in_names, out_names, out_avals, zero_outs = [], [], [], []
    for alloc in nc.m.functions[0].allocations:
        if not isinstance(alloc, _mybir.MemoryLocationSet):
            continue
        name = alloc.memorylocations[0].name
        if alloc.kind == "ExternalInput":
            if name != partition_name:
                in_names.append(name)
        elif alloc.kind == "ExternalOutput":
            shape = tuple(alloc.tensor_shape)
            dtype = _mybir.dt.np(alloc.dtype)
            out_names.append(name)
            out_avals.append(jax.core.ShapedArray(shape, dtype))
            zero_outs.append(np.zeros(shape, dtype))
    n_params = len(in_names)
    n_outs = len(out_avals)
    all_in_names = in_names + out_names + (
        [partition_name] if partition_name else [])
    donate = tuple(range(n_params, n_params + n_outs))

    def _body(*args):
        operands = list(args)
        if partition_name is not None:
            operands.append(bass2jax.partition_id_tensor())
        outs = bass2jax._bass_exec_p.bind(
            *operands,
            out_avals=tuple(out_avals),
            in_names=tuple(all_in_names),
            out_names=tuple(out_names),
            lowering_input_output_aliases=(),
            sim_require_finite=True,
            sim_require_nnan=True,
            nc=nc,
        )
        return tuple(outs)

    devices = jax.devices()[:N_CORES]
    mesh = Mesh(np.asarray(devices), ("core",))
    in_specs = (PartitionSpec("core"),) * (n_params + n_outs)
    out_specs = (PartitionSpec("core"),) * n_outs
    fn = jax.jit(
        shard_map(_body, mesh=mesh, in_specs=in_specs, out_specs=out_specs,
                  check_rep=False),
        donate_argnums=donate, keep_unused=True,
    )
    exe = {
        "fn": fn, "mesh": mesh, "in_names": in_names,
        "out_names": out_names, "out_avals": out_avals,
        "zero_outs": zero_outs, "n_params": n_params,
    }
    _CACHE["exe"] = exe
    return exe


def _concat_inputs(exe, in_maps):
    return [
        np.concatenate([np.asarray(in_maps[c][name])
                        for c in range(N_CORES)], axis=0)
        for name in exe["in_names"]
    ]


def _concat_zeros(exe):
    return [np.zeros((N_CORES * z.shape[0], *z.shape[1:]), z.dtype)
            for z in exe["zero_outs"]]


def kernel(a, h, W_proj, b_proj, w_att, b_att):
    exe = _get_executable()
    in_maps = _make_in_maps(a, h, W_proj, b_proj, w_att, b_att)
    out_arrs = exe["fn"](*_concat_inputs(exe, in_maps), *_concat_zeros(exe))
    i = exe["out_names"].index("out")
    out = np.asarray(out_arrs[i]).reshape(N_CORES, N, D).copy()
    # the kernel computes P@hp0/rs + hp0 with hp0 = h@W (bias-free); the
    # missing constant row 2*b_proj is added here (exact algebra).
    out += 2.0 * np.asarray(b_proj, dtype=np.float32).reshape(1, 1, D)
    return out


if __name__ == "__main__":
    rng = np.random.default_rng(0)
    a = rng.random((B, N, N), dtype=np.float32)
    h = rng.standard_normal((B, N, D), dtype=np.float32)
    W_proj = (rng.standard_normal((D, D)) / np.sqrt(D)).astype(np.float32)
    b_proj = (rng.standard_normal(D) * 0.01).astype(np.float32)
    w_att = (rng.standard_normal(2 * D) / np.sqrt(2 * D)).astype(np.float32)
    b_att = np.float32(rng.standard_normal() * 0.01)

    got = kernel(a=a, h=h, W_proj=W_proj, b_proj=b_proj, w_att=w_att,
                 b_att=b_att)

    hp = h @ W_proj + b_proj
    s = hp @ w_att[:D]
    t = hp @ w_att[D:]
    e = np.maximum(s[:, :, None] + t[:, None, :] + b_att, 0.0)
    att = np.exp(e) * a
    att = att / att.sum(-1, keepdims=True)
    ref = att @ hp + hp

    err = np.abs(got - ref).max() / np.abs(ref).max()
    print("rel err:", err)


# revision 22
# speedup vs baseline: 1.1071x; 1.0334x over previous
"""GAT layer kernel for Trainium2, data-parallel over batch across 8 NeuronCores.

Per batch element b (one core each):
    hp  = h @ W_proj + b_proj                      # [N, D]
    s   = hp @ w_src ; t = hp @ w_dst              # [N]
    e   = relu(s[:,None] + t[None,:] + b_att)      # [N, N]
    att = exp(e) * a ; att /= att.sum(-1, keepdim) # [N, N]
    out = att @ hp + hp                            # [N, D]

Design (v2 — transposed-domain, zero on-device transposes):
  * Host marshaling: a is shipped TRANSPOSED and cast to bf16 (aT[j,i] =
    a[i,j]); h is shipped transposed (hT f32). Tiny weights are shipped
    pre-replicated (w_src tiled to 128 cols, b_att to 128 rows).
  * exp(relu(x)) == max(exp(x), 1) and exp(s_i+t_j) == u_i*v_j with
    u = exp(s), v = exp(t + b_att). u is materialized replicated across
    partitions ([128, N] bf16) via a matmul with a column-replicated w_src;
    v lands as a per-partition scalar column ([128, 16]).
  * The score matrix is built directly in TRANSPOSED orientation, per
    128-row j-chunk: PT[j, i] = max(u_i * v_j, 1) * aT[j, i]
    (DVE tensor_scalar 4x-bf16 + tensor_tensor 2x-bf16). No DMA/PE
    transposes of the NxN matrix anywhere.
  * Matmul: PT chunks are the STATIONARY operand, rhs = [hp + b | 1]
    ([128, 129] bf16). Output accumulates NATURALLY as [i, d] in PSUM and
    the row-sums fall out as the free 129th column. 16 accumulators are
    packed 3-per-bank into 6 PSUM banks.
  * Algebra: rhs cols 0:128 carry hp + b_proj; the residual added at the
    end is hp0 = h@W (bias-free). Then
        psum/rowsum + hp0 = P@hp/rs + b_proj + hp0 = P@hp/rs + hp.  (exact)
  * Finalize: one DVE reciprocal + one fused scalar_tensor_tensor per
    i-chunk: out = psum * (1/rowsum) + hp0.
"""

import os
import sys

for _p in ("/opt/trn_rl_repo", "/root/.axon_site/_ro/trn_rl_repo"):
    if _p not in sys.path and os.path.isdir(_p):
        sys.path.append(_p)

import numpy as np
from contextlib import ExitStack

import concourse.bass as bass
import concourse.bacc as bacc
import concourse.tile as tile
from concourse import mybir
from concourse.bass_utils import run_bass_kernel_spmd

F32 = mybir.dt.float32
BF16 = mybir.dt.bfloat16
MULT = mybir.AluOpType.mult
MAX = mybir.AluOpType.max
ADD = mybir.AluOpType.add
EXP = mybir.ActivationFunctionType.Exp
LRELU = mybir.ActivationFunctionType.Lrelu
COPY = mybir.ActivationFunctionType.Copy
IDENT = mybir.ActivationFunctionType.Identity

B, N, D = 8, 2048, 128
P = 128           # partitions
NT = N // P       # 16 chunks
N_CORES = 8


def _build_kernel(ctx: ExitStack, tc: tile.TileContext, io: dict):
    nc = tc.nc
    aT = io["aT"]          # [N, N] bf16 dram: aT[j, i] = a[i, j]
    hx_d = io["hx"]        # [D, N+257] bf16: [hT | ws2r | wd2 | W] packed



    cas = io["cas"]        # [128, 1] f32: b_proj @ w_src, replicated
    ba2 = io["ba2"]        # [128, 1] f32: b_proj @ w_dst + b_att, replicated
    out = io["out"]        # [N, D] f32 dram

    cst = ctx.enter_context(tc.tile_pool(name="cst", bufs=1))
    sps = ctx.enter_context(tc.tile_pool(name="sps", bufs=2, space="PSUM"))
    mmp = ctx.enter_context(tc.tile_pool(name="mmp", bufs=1, space="PSUM"))
    zp = ctx.enter_context(tc.tile_pool(name="zp", bufs=1))
    zap = ctx.enter_context(tc.tile_pool(name="zap", bufs=1))
    ppx = ctx.enter_context(tc.tile_pool(name="ppx", bufs=1))
    rp = ctx.enter_context(tc.tile_pool(name="rp", bufs=1))

    # ---- ACT warm-up: trigger the lazy ACT_TABLE_LOAD off the critical path
    warm_in = cst.tile([1, 1], F32)
    nc.vector.memset(warm_in[:], 0.0)
    warm_out = cst.tile([1, 1], F32)
    nc.scalar.activation(warm_out[:], warm_in[:], EXP)

    # ---- critical loads on the Sync HWDGE queue, then the aT flood ----
    hx_sb = cst.tile([P, N + 257], BF16)
    nc.sync.dma_start(hx_sb[:], hx_d[:])
    hTb_sb = hx_sb[:, 0:N]
    ws2rb_sb = hx_sb[:, N:N + 128]
    wd2b_sb = hx_sb[:, N + 128:N + 129]
    Wb_sb = hx_sb[:, N + 129:N + 257]
    cas_sb = cst.tile([P, 1], F32)
    nc.sync.dma_start(cas_sb[:], cas[:])
    aT_tiles = []

    def load_aT(jc):
        a_t = cst.tile([P, N], BF16, tag=f"at{jc}", name=f"aT{jc}")
        nc.sync.dma_start(
            a_t[:],
            aT[jc * P:(jc + 1) * P, :].rearrange("(c p) i -> p c i", p=P))
        aT_tiles.append(a_t)

    for jc in range(NT):
        load_aT(jc)

    # ---- secondary loads on the Scalar HWDGE queue (parallel issue) ----
    ba2_sb = cst.tile([P, 1], F32)
    nc.scalar.dma_start(ba2_sb[:], ba2[:])

    # ---- u_full [p, i] bf16 = exp(s_i) replicated; s = h @ (W w_src) + cas
    u_full = cst.tile([P, N], BF16)
    for s4 in range(4):
        sl = slice(s4 * 512, (s4 + 1) * 512)
        ps = sps.tile([P, 512], F32, tag="sps")
        nc.tensor.matmul(ps[:], ws2rb_sb, hTb_sb[:, sl])
        nc.scalar.activation(u_full[:, sl], ps[:], EXP, bias=cas_sb[:],
                             scale=1.0)

    # ---- t/v: t[jc*128+p] via 1-wide bf16 matmuls on hTb ----
    v_col = cst.tile([P, NT], F32)
    t_ps = sps.tile([P, 512], F32, tag="sps")
    for r in range(NT):
        nc.tensor.matmul(t_ps[:, r:r + 1], hTb_sb[:, r * P:(r + 1) * P],
                         wd2b_sb)
        if r == 7:
            nc.scalar.activation(v_col[:, 0:8], t_ps[:, 0:8], EXP,
                                 bias=ba2_sb[:], scale=1.0)
    nc.scalar.activation(v_col[:, 8:NT], t_ps[:, 8:NT], EXP, bias=ba2_sb[:],
                         scale=1.0)

    # ---- hp chunks: hp0 = h @ W (f32), hp_aug = [hp0 | 1] bf16.
    # Algebra: P@hp0/rs + hp0 == P@hp/rs + hp - 2*b_proj; the constant
    # 2*b_proj row is added back on the HOST after the kernel returns. ----
    hp_aug = cst.tile([P, NT, 132], BF16)
    nc.vector.memset(hp_aug[:, :, 128:129], 1.0)

    def emit_hp(r):
        if r >= NT:
            return
        ps = sps.tile([P, 512], F32, tag="sps", name="hp_ps")
        nc.tensor.matmul(ps[:, :P], hTb_sb[:, r * P:(r + 1) * P], Wb_sb)
        nc.scalar.activation(hp_aug[:, r, 0:P], ps[:, :P], COPY)

    for r in range(NT):
        emit_hp(r)

    # ---- main psum: 16 accumulators [128, 129], packed 3 per bank ----
    mm_tiles = [mmp.tile([P, 512], F32, tag=f"mm{b}", name=f"mm{b}")
                for b in range(6)]

    def acc(ic):
        bank, slot = divmod(ic, 3)
        off = 130 * slot
        return mm_tiles[bank], off

    out_stage = cst.tile([P, NT, D], F32)

    # ---- main loop over j-chunks; z runs 3 chunks ahead of the product
    # so the pre-p0 DVE idle window does useful z work ----
    z_tiles = {}

    def emit_z(jc):
        if jc >= NT:
            return
        z_t = zp.tile([P, N], BF16, tag=f"z{jc % 3}", name="z_t")
        nc.vector.tensor_scalar(z_t[:], u_full[:], v_col[:, jc:jc + 1],
                                1.0, MULT, MAX)
        z_tiles[jc] = z_t

    for jc in range(3):
        emit_z(jc)
    for jc in range(NT):
        p_t = ppx.tile([P, N], BF16, tag=f"p{jc % 3}", name="p_t")
        nc.vector.tensor_tensor(p_t[:], z_tiles.pop(jc)[:], aT_tiles[jc][:],
                                MULT)
        emit_z(jc + 3)
        for ic in range(NT):
            mt, off = acc(ic)
            # start=True clears PSUM has_written at BANK granularity on
            # TRN2: only the first accumulator of each bank may use it, or
            # it wipes its siblings' jc=0 contribution. The bank-wide clear
            # leaves the sibling slots "unwritten", so their jc=0 matmul
            # (start=False) overwrites rather than accumulates - correct on
            # every execution.
            nc.tensor.matmul(mt[:, off:off + 129],
                             p_t[:, ic * P:(ic + 1) * P],
                             hp_aug[:, jc, 0:129],
                             start=(jc == 0 and ic % 3 == 0),
                             stop=(jc == NT - 1),
                             skip_group_check=True)

    # ---- finalize in 4 groups of 4: out = psum/rowsum + hp0.
    # Two parallel pipelines per group: 2 chunks fused on DVE (stt), 2 on
    # ACT (scaled copy) + Pool (residual add), so no engine chain
    # serializes. ----
    out_r = out.rearrange("(r p) d -> p r d", p=P)
    rinv_all = cst.tile([P, NT], F32)
    for g in range(4):
        ics = list(range(4 * g, 4 * g + 4))
        for ic in ics:
            mt, off = acc(ic)
            nc.vector.reciprocal(rinv_all[:, ic:ic + 1],
                                 mt[:, off + 128:off + 129])
        for ic in ics[2:]:
            mt, off = acc(ic)
            fin = rp.tile([P, D], F32, tag=f"fin{ic % 4}", name="fin")
            nc.scalar.activation(fin[:], mt[:, off:off + 128], COPY,
                                 scale=rinv_all[:, ic:ic + 1])
            nc.gpsimd.tensor_tensor(out_stage[:, ic, :], fin[:],
                                    hp_aug[:, ic, 0:P], ADD)
        for ic in ics[:2]:
            mt, off = acc(ic)
            nc.vector.scalar_tensor_tensor(out_stage[:, ic, :],
                                           mt[:, off:off + 128],
                                           rinv_all[:, ic:ic + 1],
                                           hp_aug[:, ic, 0:P], MULT, ADD)
        nc.sync.dma_start(out_r[:, 4 * g:4 * g + 4, :],
                          out_stage[:, 4 * g:4 * g + 4, :])


_CACHE = {}


def _get_compiled():
    if "nc" in _CACHE:
        return _CACHE["nc"], _CACHE["names"]

    nc = bacc.Bacc("TRN2", target_bir_lowering=False, debug=False)
    io = {}
    io["aT"] = nc.dram_tensor("aT", [N, N], BF16, kind="ExternalInput").ap()
    io["hx"] = nc.dram_tensor("hx", [D, N + 257], BF16, kind="ExternalInput").ap()

    io["cas"] = nc.dram_tensor("cas", [P, 1], F32, kind="ExternalInput").ap()
    io["ba2"] = nc.dram_tensor("ba2", [P, 1], F32, kind="ExternalInput").ap()
    io["out"] = nc.dram_tensor("out", [N, D], F32, kind="ExternalOutput").ap()

    with tile.TileContext(nc) as tc:
        with ExitStack() as ctx:
            _build_kernel(ctx, tc, io)
    nc.compile()

    _CACHE["nc"] = nc
    _CACHE["names"] = list(io.keys())
    return nc, _CACHE["names"]


def _make_in_maps(a, h, W_proj, b_proj, w_att, b_att):
    import ml_dtypes
    bf16 = ml_dtypes.bfloat16

    a = np.asarray(a, dtype=np.float32)
    h = np.asarray(h, dtype=np.float32)
    W_proj = np.ascontiguousarray(W_proj, dtype=np.float32)
    b_proj = np.asarray(b_proj, dtype=np.float32).reshape(D)
    w_att = np.ascontiguousarray(w_att, dtype=np.float32)
    w_src, w_dst = w_att[:D], w_att[D:]

    ws2 = (W_proj @ w_src).astype(np.float32)
    ws2rb = np.tile(ws2[:, None], (1, P)).astype(bf16)
    wd2b = (W_proj @ w_dst).astype(np.float32).reshape(D, 1).astype(bf16)
    cas = np.full((P, 1), float(b_proj @ w_src), dtype=np.float32)
    ba2 = np.full((P, 1), float(b_proj @ w_dst) + float(b_att),
                  dtype=np.float32)

    in_maps = []
    for c in range(N_CORES):
        hT_c = np.ascontiguousarray(h[c].T)
        in_maps.append({
            "aT": np.ascontiguousarray(a[c].T).astype(bf16),
            "hx": np.ascontiguousarray(np.concatenate(
                [hT_c.astype(bf16), ws2rb, wd2b, W_proj.astype(bf16)],
                axis=1)),
            "cas": cas, "ba2": ba2,
        })
    return in_maps


def _get_executable():
    """Build (once) a sharded PJRT callable for the compiled Bass module.

    Mirrors concourse.bass2jax.run_bass_via_pjrt but keeps the jitted
    function so repeated calls don't retrace/recompile.
    """
    if "exe" in _CACHE:
        return _CACHE["exe"]

    import jax
    from jax.sharding import Mesh, PartitionSpec
    from jax.experimental.shard_map import shard_map
    from concourse import bass2jax, mybir as _mybir

    nc, _ = _get_compiled()
    bass2jax.install_neuronx_cc_hook()

    partition_name = (nc.partition_id_tensor.name
                      if nc.partition_id_tensor else None)
    in_names, out_names, out_avals, zero_outs = [], [], [], []
    for alloc in nc.m.functions[0].allocations:
        if not isinstance(alloc, _mybir.MemoryLocationSet):
            continue
        name = alloc.memorylocations[0].name
        if alloc.kind == "ExternalInput":
            if name != partition_name:
                in_names.append(name)
        elif alloc.kind == "ExternalOutput":
            shape = tuple(alloc.tensor_shape)
            dtype = _mybir.dt.np(alloc.dtype)
            out_names.append(name)
            out_avals.append(jax.core.ShapedArray(shape, dtype))
            zero_outs.append(np.zeros(shape, dtype))
    n_params = len(in_names)
    n_outs = len(out_avals)
    all_in_names = in_names + out_names + (
        [partition_name] if partition_name else [])
    donate = tuple(range(n_params, n_params + n_outs))

    def _body(*args):
        operands = list(args)
        if partition_name is not None:
            operands.append(bass2jax.partition_id_tensor())
        outs = bass2jax._bass_exec_p.bind(
            *operands,
            out_avals=tuple(out_avals),
            in_names=tuple(all_in_names),
            out_names=tuple(out_names),
            lowering_input_output_aliases=(),
            sim_require_finite=True,
            sim_require_nnan=True,
            nc=nc,
        )
        return tuple(outs)

    devices = jax.devices()[:N_CORES]
    mesh = Mesh(np.asarray(devices), ("core",))
    in_specs = (PartitionSpec("core"),) * (n_params + n_outs)
    out_specs = (PartitionSpec("core"),) * n_outs
    fn = jax.jit(
        shard_map(_body, mesh=mesh, in_specs=in_specs, out_specs=out_specs,
                  check_rep=False),
        donate_argnums=donate, keep_unused=True,
    )
    exe = {
        "fn": fn, "mesh": mesh, "in_names": in_names,
        "out_names": out_names, "out_avals": out_avals,
        "zero_outs": zero_outs, "n_params": n_params,
    }
    _CACHE["exe"] = exe
    return exe


def _concat_inputs(exe, in_maps):
    return [
        np.concatenate([np.asarray(in_maps[c][name])
                        for c in range(N_CORES)], axis=0)
        for name in exe["in_names"]
    ]


def _concat_zeros(exe):
    return [np.zeros((N_CORES * z.shape[0], *z.shape[1:]), z.dtype)
            for z in exe["zero_outs"]]


def kernel(a, h, W_proj, b_proj, w_att, b_att):
    exe = _get_executable()
    in_maps = _make_in_maps(a, h, W_proj, b_proj, w_att, b_att)
    out_arrs = exe["fn"](*_concat_inputs(exe, in_maps), *_concat_zeros(exe))
    i = exe["out_names"].index("out")
    out = np.asarray(out_arrs[i]).reshape(N_CORES, N, D).copy()
    # the kernel computes P@hp0/rs + hp0 with hp0 = h@W (bias-free); the
    # missing constant row 2*b_proj is added here (exact algebra).
    out += 2.0 * np.asarray(b_proj, dtype=np.float32).reshape(1, 1, D)
    return out


if __name__ == "__main__":
    rng = np.random.default_rng(0)
    a = rng.random((B, N, N), dtype=np.float32)
    h = rng.standard_normal((B, N, D), dtype=np.float32)
    W_proj = (rng.standard_normal((D, D)) / np.sqrt(D)).astype(np.float32)
    b_proj = (rng.standard_normal(D) * 0.01).astype(np.float32)
    w_att = (rng.standard_normal(2 * D) / np.sqrt(2 * D)).astype(np.float32)
    b_att = np.float32(rng.standard_normal() * 0.01)

    got = kernel(a=a, h=h, W_proj=W_proj, b_proj=b_proj, w_att=w_att,
                 b_att=b_att)

    hp = h @ W_proj + b_proj
    s = hp @ w_att[:D]
    t = hp @ w_att[D:]
    e = np.maximum(s[:, :, None] + t[:, None, :] + b_att, 0.0)
    att = np.exp(e) * a
    att = att / att.sum(-1, keepdims=True)
    ref = att @ hp + hp

    err = np.abs(got - ref).max() / np.abs(ref).max()
    print("rel err:", err)
